# revision 50
# baseline (speedup 1.0000x reference)
"""RNN-T decoder kernel for TRN2 (8 cores, T-sharded joint, replicated LSTM).

v2: overlap-restructured.

Layout notes
------------
B=8, T=128, U=64, E=512, H=1024 (8 k-chunks), J=640 (5 j-chunks), OD=1024.
Each core handles T-slice [16c, 16c+16) of the joint; the 2-layer LSTM over U
is computed identically (replicated, all 8 batches) on every core.

Gate permutation: hidden dim is split in 4 quarters (col-tile groups). Group
j's 1024 gate columns are [i_j | f_j | o_j | g_j] (256 each), where x_j acts
on hidden units [256j, 256j+256). Weights/bias/X tensors are host-permuted
to this order.

Gates PSUM tile (128, 1024): group j occupies partitions [32j, 32j+8)
(batch-major), accumulated by 4-way column-packed matmuls (tile_position).

v2 changes vs baseline:
- X inject is a DVE add (grouped-layout xg tiles), not PE matmuls.
- h transposes go through the DMA xbar (sync engine), not the PE.
- X0 weights SBUF-resident; X1 weights ring-prefetched.
- Joint is computed per-16-u chunk and interleaved into the LSTM wavefronts.
- Joint/out in fp16 (output upcast on host).
"""
import numpy as np
import ml_dtypes

import concourse.bass as bass
import concourse.bacc as bacc
import concourse.mybir as mybir
import concourse.tile as tile

dt = mybir.dt
AF = mybir.ActivationFunctionType

B, T, E, H, J, OD = 8, 128, 512, 1024, 640, 1024
HK = H // 128   # 8 h-chunks
JC = J // 128   # 5 j-chunks
EK = E // 128   # 4 e-chunks
TSH = T // 8    # 16 t per core
NG = 4          # col-tile groups

W1_RING = 2     # wih1 chunk ring depth (4KB/part each)
XG_BUFS = 2


def build_program(U=64, n_cores=8, with_biases=False,
                  with_out_bias=False):
    nc = bacc.Bacc("TRN2", target_bir_lowering=False, debug=False,
                   num_devices=n_cores)
    f16, f32, i32 = dt.float16, dt.float32, dt.int32
    UG = U // 16  # u-blocks of 16
    assert U % 16 == 0

    # ---------------- external inputs ----------------
    embed_d = nc.dram_tensor("embed", [OD, E], f32, kind="ExternalInput")
    yidx_d = nc.dram_tensor("yidx", [128, B * U // 128], i32, kind="ExternalInput")
    wih0_d = nc.dram_tensor("wih0t", [128, EK, 4096], f16, kind="ExternalInput")
    wih1_d = nc.dram_tensor("wih1t", [16, 128, HK, 256], f16, kind="ExternalInput")
    whh0_d = nc.dram_tensor("whh0t", [128, HK, NG, 1024], f16, kind="ExternalInput")
    whh1_d = nc.dram_tensor("whh1t", [128, HK, NG, 1024], f16, kind="ExternalInput")
    eye128_d = nc.dram_tensor("eye128", [128, 128], f16, kind="ExternalInput")
    injrep_d = nc.dram_tensor("injrep", [128, 8], f16, kind="ExternalInput")
    wenc_d = nc.dram_tensor("wenct", [JC, 128, HK, 128], f16, kind="ExternalInput")
    wdec_d = nc.dram_tensor("wdect", [JC, 128, HK, 128], f16, kind="ExternalInput")
    wout_d = nc.dram_tensor("woutt", [128, JC, OD], f16, kind="ExternalInput")
    benc_d = nc.dram_tensor("benc", [128, JC], f32, kind="ExternalInput")
    bout_d = nc.dram_tensor("boutrep", [128, OD], f32, kind="ExternalInput")
    hst_d = nc.dram_tensor("hst16", [128, HK, B * TSH], f16, kind="ExternalInput")
    # per-layer (b_ih + b_hh), gate-permuted, replicated over partitions
    bi0_d = nc.dram_tensor("bihh0", [128, 4096], f16, kind="ExternalInput")
    bi1_d = nc.dram_tensor("bihh1", [128, 4096], f16, kind="ExternalInput")

    out_d = nc.dram_tensor("out", [B * TSH * U, OD], f16, kind="ExternalOutput")

    # ---------------- internal dram ----------------
    # grouped X layout: [u, group j, row r (r<8 = batch b, rest junk), 1024]
    x0_d = nc.dram_tensor("X0d", [U, NG, 32, 1024], f16)
    x1_d = nc.dram_tensor("X1d", [U, NG, 32, 1024], f16)

    with tile.TileContext(nc) as tc:
        with (
            tc.tile_pool(name="const", bufs=1) as pc,
            tc.tile_pool(name="lstmS", bufs=1) as lS,
            tc.tile_pool(name="lstmPS", bufs=1, space="PSUM") as lP,
        ):
            # h_dec transposed history, both layers (fp16)
            hdec = [pc.tile([128, HK, U, B], f16, tag=f"hdec{l}",
                            name=f"hdec{l}") for l in range(2)]
            if with_biases:
                bi0_sb = pc.tile([128, 4096], f16, tag="bi0")
                nc.gpsimd.dma_start(bi0_sb[:], bi0_d.ap())
                bi1_sb = pc.tile([128, 4096], f16, tag="bi1")
                nc.gpsimd.dma_start(bi1_sb[:], bi1_d.ap())

            # recurrent weights (resident for whole LSTM); whh1 gets its own
            # pool opened at w8 (after the prologue pool frees its space)
            whh_sb = [pc.tile([128, HK, NG, 1024], f16, tag="whh0",
                              name="whh0"), None]

            # yidx first: the gather (prologue critical path) waits it
            yidx_sb = pc.tile([128, B * U // 128], i32, tag="yidx")
            nc.sync.dma_start(yidx_sb[:], yidx_d.ap())
            # joint resident tiles (DMAs for wout emitted later)
            wout_sb = pc.tile([128, JC, OD], f16, tag="wout")
            ze_sb = pc.tile([128, JC, B * TSH], f16, tag="ze")
            benc_sb = pc.tile([128, JC], f32, tag="bencs")
            nc.sync.dma_start(benc_sb[:], benc_d.ap())
            if with_out_bias:
                bout_sb = pc.tile([128, OD], f32, tag="bouts")
                nc.sync.dma_start(bout_sb[:], bout_d.ap())
            eye128_sb = pc.tile([128, 128], f16, tag="eye128")
            nc.sync.dma_start(eye128_sb[:], eye128_d.ap())
            injrep_sb = pc.tile([128, 8], f16, tag="injrep")
            nc.sync.dma_start(injrep_sb[:], injrep_d.ap())

            # ---------------- prologue pool (released at w8) ----------------
            # yidx on the fast sync queue; the embedding gather's indirect
            # DMA must be at the head of the gpsimd queue so the prologue
            # critical path (gather -> X0 block 0 -> w00) starts promptly
            NCH = B * U // 128  # row chunks of 128 (one per u-block of 16)
            pre_ctx = tc.tile_pool(name="preS", bufs=1)
            pS = pre_ctx.__enter__()
            hst_sb = pc.tile([128, HK, B * TSH], f16, tag="hst")
            nc.sync.dma_start(hst_sb[:], hst_d.ap())
            wih0_sb = pS.tile([128, EK, 4096], f16, tag="wih0")
            nc.scalar.dma_start(wih0_sb[:], wih0_d.ap())
            eyst = pS.tile([128, EK, B * U], f16, tag="eyst")

            def gather_chunk(ch):
                g32 = pS.tile([128, E], f32, tag="g32", bufs=2)
                nc.gpsimd.indirect_dma_start(
                    out=g32[:], out_offset=None, in_=embed_d.ap(),
                    in_offset=bass.IndirectOffsetOnAxis(
                        ap=yidx_sb[:, ch:ch + 1], axis=0))
                g16 = pS.tile([128, E], f16, tag="g16", bufs=2)
                nc.vector.tensor_copy(g16[:], g32[:])
                for ec in range(EK):
                    tp = lP.tile([128, 128], f16, tag="outps", bufs=2)
                    nc.tensor.transpose(
                        tp[:], g16[:, ec * 128:(ec + 1) * 128], eye128_sb[:])
                    nc.vector.tensor_copy(
                        eyst[:, ec, ch * 128:(ch + 1) * 128], tp[:])

            def x0_block(g):
                # rows = (16 u, 8 b) of u-block g; full 4096 gate cols
                for j0 in range(NG):
                    x0cw = lS.tile([128, 1024], f16, tag="x1c", bufs=2)
                    for q in range(2):
                        nc_ = j0 * 2 + q
                        ps = lP.tile([128, 512], f32, tag="outps", bufs=2)
                        for ec in range(EK):
                            nc.tensor.matmul(
                                ps[:],
                                eyst[:, ec, g * 128:(g + 1) * 128],
                                wih0_sb[:, ec, nc_ * 512:(nc_ + 1) * 512],
                                start=(ec == 0), stop=(ec == EK - 1))
                        sl = slice(q * 512, (q + 1) * 512)
                        if with_biases:
                            nc.vector.tensor_add(
                                x0cw[:, sl], ps[:],
                                bi0_sb[:, nc_ * 512:(nc_ + 1) * 512])
                        else:
                            nc.vector.tensor_copy(x0cw[:, sl], ps[:])
                    nc.gpsimd.dma_start(
                        x0_d.ap()[g * 16:(g + 1) * 16, j0, 0:8, :],
                        x0cw[:])

            def ze_jc(jc):
                wec = lS.tile([128, HK, 128], f16, tag="wdc", bufs=2)
                nc.sync.dma_start(wec[:], wenc_d.ap()[jc])
                zp = lP.tile([128, 512], f32, tag="mmps", bufs=2)
                for ec in range(HK):
                    nc.tensor.matmul(zp[:, 0:128], wec[:, ec, :],
                                     hst_sb[:, ec, :],
                                     start=(ec == 0), stop=(ec == HK - 1))
                nc.scalar.activation(ze_sb[:, jc, :], zp[:, 0:128],
                                     AF.Identity,
                                     bias=benc_sb[:, jc:jc + 1])

            # ---------------- LSTM pieces ----------------
            gate_ps = [lP.tile([128, 1024], f32, tag=f"gates{l}",
                               name=f"gates{l}") for l in range(2)]
            czero = [lS.tile([128, 256], f32, tag=f"c{l}", name=f"cz{l}",
                             bufs=2) for l in range(2)]
            nc.gpsimd.memset(czero[0][:], 0.0)
            nc.gpsimd.memset(czero[1][:], 0.0)
            cprev = [czero[0], czero[1]]
            xsrc = [x0_d, x1_d]

            def lstm_mm(l, u):
                pg = gate_ps[l]
                # grouped X fetch (prefetchable; no recurrent dep)
                xg = lS.tile([128, 1024], f16, tag=f"xg{l}", bufs=XG_BUFS)
                nc.gpsimd.dma_start(xg[:], xsrc[l].ap()[u])
                if u > 0:
                    for kc in range(HK):
                        for hf in range(2):
                            sl = slice(hf * 512, (hf + 1) * 512)
                            for j in range(NG):
                                nc.tensor.matmul(
                                    pg[32 * j:32 * j + 8, sl],
                                    hdec[l][:, kc, u - 1, :],
                                    whh_sb[l][:, kc, j,
                                              hf * 512:(hf + 1) * 512],
                                    tile_position=(0, 32 * j),
                                    start=(kc == 0), stop=False)
                # x inject: diagonal-tiled eye-matmuls reading the grouped
                # xg slice for each group straight from SBUF partitions
                # 32j..32j+8 (ends each region's accumulation group);
                # validated standalone on HW (test_diag.py)
                for hf in range(2):
                    sl = slice(hf * 512, (hf + 1) * 512)
                    for j in range(NG):
                        nc.tensor.matmul(
                            pg[32 * j:32 * j + 8, sl],
                            injrep_sb[32 * j:32 * j + 8, :],
                            xg[32 * j:32 * j + 8, sl],
                            tile_position=(32 * j, 32 * j),
                            start=(u == 0), stop=True)
                return xg

            def lstm_act1(l, u, xg):
                pg = gate_ps[l]
                sig = lS.tile([128, 768], f16, tag=f"sig{l}")
                nc.scalar.activation(sig[:], pg[:, 0:768], AF.Sigmoid)
                tg = lS.tile([128, 256], f16, tag=f"tg{l}")
                nc.scalar.activation(tg[:], pg[:, 768:1024], AF.Tanh)
                return sig, tg

            def lstm_cmul(l, u, sig, tg):
                t1 = lS.tile([128, 256], f16, tag=f"t1{l}")
                nc.vector.tensor_mul(t1[:], sig[:, 0:256], tg[:])
                cnew = lS.tile([128, 256], f32, tag=f"c{l}", bufs=2)
                nc.vector.tensor_mul(cnew[:], sig[:, 256:512], cprev[l][:])
                nc.vector.tensor_add(cnew[:], cnew[:], t1[:])
                cprev[l] = cnew
                return cnew

            def lstm_tail(l, u, sig, cnew):
                tc_ = lS.tile([128, 256], f16, tag=f"tc{l}")
                nc.scalar.activation(tc_[:], cnew[:], AF.Tanh)
                h = lS.tile([128, 256], f16, tag=f"h{l}", bufs=2)
                nc.vector.tensor_mul(h[:], sig[:, 512:768], tc_[:])
                # PE transpose into a scratch corner of this layer's gates
                # PSUM bank (free between the ACT reads of step u and the
                # matmuls of step u+1) -- costs no extra PSUM banks. The
                # f16 view of the f32 gates tile keeps the transpose at
                # 1 cyc/row.
                pgb = gate_ps[l][:].bitcast(f16)  # (128, 2048)
                for cb in range(2):
                    nc.tensor.transpose(
                        pgb[:, cb * 128:(cb + 1) * 128],
                        h[:, cb * 128:(cb + 1) * 128], eye128_sb[:])
                    hd = hdec[l][:, 0, u, :]  # (128, B) at kc=0
                    dst = bass.AP(hd.tensor, hd.offset + cb * U * B,
                                  [hd.ap[0], [2 * U * B, NG], [1, B]])
                    src_ap = pgb[:, cb * 128:(cb + 1) * 128].rearrange(
                        "p (j r) -> p j r", j=NG)[:, :, 0:B]
                    nc.vector.tensor_copy(dst, src_ap)

            def x1_block(kb):
                hd0 = hdec[0]
                for grp in range(4):
                    x1cw = lS.tile([128, 1024], f16, tag="x1c", bufs=2)
                    for q in range(4):
                        nc2 = grp * 4 + q
                        w1c = lS.tile([128, HK, 256], f16, tag="w1c",
                                      bufs=W1_RING)
                        nc.sync.dma_start(w1c[:], wih1_d.ap()[nc2])
                        ps = lP.tile([128, 512], f32, tag="mmps", bufs=2)
                        for kc in range(HK):
                            nc.tensor.matmul(
                                ps[:, 0:256],
                                hd0[:, kc, kb * 16:(kb + 1) * 16, :],
                                w1c[:, kc, :],
                                start=(kc == 0), stop=(kc == HK - 1))
                        sl = slice(q * 256, (q + 1) * 256)
                        if with_biases:
                            nc.vector.tensor_add(
                                x1cw[:, sl], ps[:, 0:256],
                                bi1_sb[:, nc2 * 256:(nc2 + 1) * 256])
                        else:
                            nc.vector.tensor_copy(x1cw[:, sl], ps[:, 0:256])
                    nc.gpsimd.dma_start(
                        x1_d.ap()[kb * 16:(kb + 1) * 16, grp, 0:8, :],
                        x1cw[:])

            def zd_chunk(k):
                # zd for u in [8k, 8k+8): (J-part, u, b), wdec streamed
                zdt = pc.tile([128, JC, 8, B], f32, tag="zd", bufs=2)
                for jc in range(JC):
                    wdc = lS.tile([128, HK, 128], f16, tag="wdc", bufs=2)
                    nc.sync.dma_start(wdc[:], wdec_d.ap()[jc])
                    zp = lP.tile([128, 512], f32, tag="mmps", bufs=2)
                    for kc in range(HK):
                        nc.tensor.matmul(
                            zp[:, 0:64], wdc[:, kc, :],
                            hdec[1][:, kc, 8 * k:8 * (k + 1), :]
                            .rearrange("p u b -> p (u b)"),
                            start=(kc == 0), stop=(kc == HK - 1))
                    nc.vector.tensor_copy(
                        zdt[:, jc, :, :].rearrange("p u b -> p (u b)"),
                        zp[:, 0:64])
                return zdt

            def joint_stage1(k, b, zdt):
                # 128 rows = 16 tl pairs (batch b) x 8 u (u in chunk k).
                # per-jc ops: small quanta so the LSTM chain's ACT/DVE ops
                # are not delayed behind a long-running one
                zjt = lS.tile([128, JC, 128], f16, tag="zjt", bufs=2)
                zj = lS.tile([128, JC, 128], f16, tag="zj", bufs=2)
                for jc in range(JC):
                    ze_bc = ze_sb[:, jc, b * TSH:(b + 1) * TSH].to_broadcast(
                        (128, TSH, 8))
                    zdv = zdt[:, jc, :, b]  # (128, 8) stride B
                    zd_bc = bass.AP(zdv.tensor, zdv.offset,
                                    [zdv.ap[0], [0, TSH], zdv.ap[1]])
                    nc.vector.tensor_tensor(
                        zjt[:, jc, :].rearrange("p (a u) -> p a u", a=TSH),
                        ze_bc, zd_bc, op=mybir.AluOpType.add)
                    nc.scalar.activation(zj[:, jc, :], zjt[:, jc, :],
                                         AF.Tanh)
                return zj

            def joint_stage2(zj, k, b):
                osb = lS.tile([128, OD], f16, tag="osb", bufs=2)
                for n2 in range(2):
                    ops_ = lP.tile([128, 512], f32, tag="outps", bufs=2)
                    for jc in range(JC):
                        nc.tensor.matmul(
                            ops_[:],
                            zj[:, jc, :],
                            wout_sb[:, jc, n2 * 512:(n2 + 1) * 512],
                            start=(jc == 0), stop=(jc == JC - 1))
                    if with_out_bias:
                        nc.vector.tensor_add(
                            osb[:, n2 * 512:(n2 + 1) * 512], ops_[:],
                            bout_sb[:, n2 * 512:(n2 + 1) * 512])
                    else:
                        nc.vector.tensor_copy(
                            osb[:, n2 * 512:(n2 + 1) * 512], ops_[:])
                # out rows: b*TSH*U + tl*U + u, tl in [0, 16), u in chunk k
                nc.gpsimd.dma_start(
                    out_d.ap().rearrange("(b tl u) od -> b tl u od",
                                         b=B, tl=TSH)[
                        b, :, 8 * k:8 * (k + 1), :],
                    osb[:])

            # ---- emission: wavefronts with everything interleaved ----
            with nc.named_scope("gat0"):
                gather_chunk(0)
            # whh0 load behind the gather on gpsimd (needed from w01)
            nc.gpsimd.dma_start(whh_sb[0][:], whh0_d.ap())
            with nc.named_scope("x0b0"):
                x0_block(0)

            jq = []        # pending joint blocks (k, blk, zd tile)
            pending2 = []  # [(zj, k, blk)] awaiting stage2
            # zd chunk k is ready after wavefront 23+8k; nudge off the x1
            # burst wavefronts (w % 16 == 15, w < 79)
            zd_at = {}
            for k in range(U // 8):
                zd_at.setdefault(24 + 8 * k, []).append(k)
            for w in range(U + 17):
                steps = []
                if w < U:
                    steps.append((0, w))
                if w >= 17:
                    steps.append((1, w - 17))
                with nc.named_scope(f"w{w:02d}"):
                    xgs = {}
                    for l, u in steps:
                        xgs[l] = lstm_mm(l, u)
                # high-priority PE fillers: joint out-matmuls + x1 burst
                for zj, k, blk in pending2:
                    with nc.named_scope(f"jb{k}_{blk}"):
                        joint_stage2(zj, k, blk)
                pending2 = []
                # the serial LSTM chain (act -> cmul -> tail)
                with nc.named_scope(f"c{w:02d}"):
                    st = {}
                    for l, u in steps:
                        st[l] = lstm_act1(l, u, xgs[l])
                    cn = {}
                    for l, u in steps:
                        cn[l] = lstm_cmul(l, u, *st[l])
                    for l, u in steps:
                        lstm_tail(l, u, st[l][0], cn[l])
                # x1 burst MUST be emitted after this wavefront's scatter
                # (it reads hdec0 up to and including this wavefront's u)
                if w % 16 == 15 and (w - 15) // 16 < UG:
                    with nc.named_scope(f"x1b{(w - 15) // 16}"):
                        x1_block((w - 15) // 16)
                # prologue fillers early on
                if 1 <= w <= 3:
                    with nc.named_scope(f"gat{w}"):
                        gather_chunk(w)
                if 4 <= w <= 6 and w - 3 < UG:
                    with nc.named_scope(f"x0b{w - 3}"):
                        x0_block(w - 3)
                if 12 <= w <= 16:
                    with nc.named_scope(f"ze{w - 12}"):
                        ze_jc(w - 12)
                if w == 8:
                    pre_ctx.__exit__(None, None, None)
                    pw1_ctx = tc.tile_pool(name="whh1p", bufs=1)
                    pw1 = pw1_ctx.__enter__()
                    whh_sb[1] = pw1.tile([128, HK, NG, 1024], f16,
                                         tag="whh1", name="whh1")
                    nc.sync.dma_start(whh_sb[1][:], whh1_d.ap())
                if w == 10:
                    nc.gpsimd.dma_start(wout_sb[:], wout_d.ap())
                # stage1 of the next joint blocks (low priority this
                # wavefront: their ACT/DVE ops rank below the LSTM chain)
                while jq and len(pending2) < 2:
                    k, blk, zdt = jq.pop(0)
                    with nc.named_scope(f"js{k}_{blk}"):
                        pending2.append((joint_stage1(k, blk, zdt), k, blk))
                # zd once layer-1 u-sub-block done (lowest priority: its
                # matmuls are pure filler and must not head-of-line block
                # the next wavefront's packs)
                for k in zd_at.get(w, []):
                    with nc.named_scope(f"zd{k}"):
                        zdt = zd_chunk(k)
                    jq.extend((k, blk, zdt) for blk in range(8))
            # tail: remaining joint blocks
            while jq or pending2:
                for zj, k, blk in pending2:
                    with nc.named_scope(f"jb{k}_{blk}"):
                        joint_stage2(zj, k, blk)
                pending2 = []
                while jq and len(pending2) < 2:
                    k, blk, zdt = jq.pop(0)
                    with nc.named_scope(f"js{k}_{blk}"):
                        pending2.append((joint_stage1(k, blk, zdt), k, blk))
            pw1_ctx.__exit__(None, None, None)

    nc.compile()
    return nc


# ---------------- host-side prep ----------------

def gate_perm():
    """perm[j*1024 + s] -> row index in torch (i,f,g,o) 4H gate layout,
    with group-local order [i|f|o|g]."""
    perm = np.zeros(4 * H, dtype=np.int64)
    for j in range(NG):
        base = j * 1024
        hid = np.arange(256) + j * 256
        perm[base + 0:base + 256] = 0 * H + hid      # i
        perm[base + 256:base + 512] = 1 * H + hid    # f
        perm[base + 512:base + 768] = 3 * H + hid    # o
        perm[base + 768:base + 1024] = 2 * H + hid   # g
    return perm


def prep_inputs(hs_pad, ys_in_pad, embed, W_ih0, W_hh0, b_ih0, b_hh0,
                W_ih1, W_hh1, b_ih1, b_hh1, W_enc, b_enc, W_dec, W_out, b_out,
                U=64, n_cores=8):
    perm = gate_perm()

    def wiht(W, KD, KC):  # (4H, KD) -> (128, KC, 4096) fp16, permuted gates
        Wp = W[perm]                      # (4096, KD)
        return np.ascontiguousarray(
            Wp.T.reshape(KC, 128, 4096).transpose(1, 0, 2)).astype(np.float16)

    def whht(W):  # (4H, H) -> (128, HK, NG, 1024) fp16
        Wp = W[perm]                      # (4096, 1024) rows=permuted gates
        # [p, kc, j, n] = Wp[j*1024+n, kc*128+p]
        a = Wp.T.reshape(HK, 128, NG, 1024).transpose(1, 0, 2, 3)
        return np.ascontiguousarray(a).astype(np.float16)

    ins = {}
    ins["embed"] = np.asarray(embed, np.float32)
    ys = np.asarray(ys_in_pad).astype(np.int32)   # (B, U)
    NCH = B * U // 128
    yy = np.zeros((128, NCH), np.int32)
    for ch in range(NCH):
        p = np.arange(128)
        yy[:, ch] = ys[p % 8, ch * 16 + p // 8]
    ins["yidx"] = yy
    ins["wih0t"] = wiht(W_ih0, E, EK)
    w1 = wiht(W_ih1, H, HK)  # (128, HK, 4096)
    ins["wih1t"] = np.ascontiguousarray(
        w1.reshape(128, HK, 16, 256).transpose(2, 0, 1, 3))
    ins["whh0t"] = whht(W_hh0)
    ins["whh1t"] = whht(W_hh1)
    ins["eye128"] = np.eye(128, dtype=np.float16)
    inj = np.zeros((128, 8), np.float16)
    for j in range(NG):
        inj[32 * j:32 * j + 8] = np.eye(8, dtype=np.float16)
    ins["injrep"] = inj
    # [p, ec, jc, m] = W[jc*128+m, ec*128+p]
    def wjt(W, KC):
        a = W.T.reshape(KC, 128, JC, 128).transpose(2, 1, 0, 3)
        return np.ascontiguousarray(a).astype(np.float16)
    ins["wenct"] = wjt(W_enc, HK)
    ins["wdect"] = wjt(W_dec, HK)
    # [p, jc, od] = W_out[od, jc*128+p]
    ins["woutt"] = np.ascontiguousarray(
        W_out.T.reshape(JC, 128, OD).transpose(1, 0, 2)).astype(np.float16)
    ins["benc"] = np.ascontiguousarray(
        b_enc.reshape(JC, 128).T).astype(np.float32)
    ins["boutrep"] = np.tile(np.asarray(b_out, np.float32)[None, :], (128, 1))
    ins["bihh0"] = np.tile(((b_ih0 + b_hh0)[perm]).astype(np.float16)[None, :],
                           (128, 1))
    ins["bihh1"] = np.tile(((b_ih1 + b_hh1)[perm]).astype(np.float16)[None, :],
                           (128, 1))

    maps = []
    for c in range(n_cores):
        m = dict(ins)
        # [p, ec, r] = hs[b, TSH*c + tl, ec*128+p], r = b*TSH+tl
        sl = np.asarray(hs_pad[:, TSH * c:TSH * (c + 1), :], np.float32)
        a = sl.reshape(B * TSH, HK, 128).transpose(2, 1, 0)
        m["hst16"] = np.ascontiguousarray(a).astype(np.float16)
        maps.append(m)
    return maps


def gather_output(results):
    outs = [np.asarray(r["out"], np.float32).reshape(B, TSH, -1, OD)
            for r in results]
    return np.concatenate(outs, axis=1)


# ---------------- entry point ----------------
import sys as _sys
import types as _types

# Recreate the missing antenv.axon_hooks so trace=True works under axon
# (used only when BASS_TRACE=1 is set by a profiling harness).
if "antenv.axon_hooks" not in _sys.modules:
    _m = _types.ModuleType("antenv.axon_hooks")

    def _get_hook():
        try:
            from trn_agent_boot.trn_boot import _ntff_profile_via_ctypes
            return _ntff_profile_via_ctypes("/opt/axon/libaxon_pjrt.so")
        except Exception:
            return None
    _m.get_axon_ntff_profile_hook = _get_hook
    _sys.modules["antenv.axon_hooks"] = _m

_NC = None
last_results = None


def kernel(**inputs):
    """Full-input RNN-T decoder: returns (B, T, U, ODIM) float32."""
    global _NC, last_results
    from concourse.bass_utils import run_bass_kernel_spmd
    U = int(np.asarray(inputs["ys_in_pad"]).shape[1])
    wb = any(float(np.abs(np.asarray(inputs[k])).max()) != 0.0
             for k in ("b_ih0", "b_hh0", "b_ih1", "b_hh1"))
    wob = float(np.abs(np.asarray(inputs["b_out"])).max()) != 0.0
    if _NC is None:
        _NC = build_program(U=U, n_cores=8, with_biases=wb, with_out_bias=wob)
    maps = prep_inputs(**inputs, U=U)
    res = run_bass_kernel_spmd(_NC, maps, core_ids=list(range(8)))
    last_results = res
    return gather_output(res.results)
